# revision 2
# baseline (speedup 1.0000x reference)
"""MoE regressor (E=16, H=1024, B=4096, top-2) on 8 trn2 NeuronCores.

Expert-parallel, count-aware schedule: the host computes top-2 routing
(replicated router, fp32), sorts experts by token count, and assigns the
8 largest as phase-0 (one per core) and the 8 smallest as phase-1.
Token embeddings are gathered, transposed to [H, C] and cast to bf16 on
the host; weights are cast to bf16 and laid out as matmul lhsT blocks.

Per-core kernel structure (v2):
- Chunks are equal-width (<=512, >=~256) so LDWEIGHTS (~107ns) always
  hides under the matmul column stream.
- Chunk-major passes: for each chunk, run all 8 m-blocks; this lets the
  first chunk start as soon as its embedding slice lands instead of
  waiting for the whole phase.
- Warm-up matmuls on a dummy tile fill the DMA dead-zone at kernel
  start so the PE HAM clock-gate reaches 2.4 GHz before real work.
- DMA triggers are ordered so the PE consumption schedule is fed just
  in time: w1[m0] -> consts -> emb chunk0 -> w1[m1] -> w1[m2] ->
  emb chunk1 -> w1[m3..7] -> phase1 weights -> phase1 embeddings.
- Second layer: DVE accumulates partial[k,c] += w2[m*128+k]*h[k,m,c]
  for m<7; a ones-vector matmul reduces over partitions fused with the
  m=7 direct matmul in one PSUM group; ScalarE drains PSUM with the
  output bias fused (Identity activation).

Self-contained: hardcodes all shapes.
"""

import numpy as np
import ml_dtypes

import concourse.bass as bass  # noqa: F401
from concourse import bacc
import concourse.mybir as mybir
import concourse.tile as tile
from concourse.bass_utils import run_bass_kernel_spmd

P = 128
B = 4096
H = 1024
E = 16
NCORES = 8
NPH = 2  # phases (experts) per core

F32 = mybir.dt.float32
BF16 = mybir.dt.bfloat16
BF_NP = ml_dtypes.bfloat16

_CACHE = {}


def _chunks(C):
    """Split C columns into equal-width chunks (<=512 each).

    Equal widths keep every chunk >=~256 wide so the 128-col LDWEIGHTS
    (~107ns) stays hidden under the column stream (cw/2.4 ns).
    """
    n = (C + 511) // 512
    base = C // n
    rem = C % n
    out = []
    c0 = 0
    for i in range(n):
        w = base + (1 if i < rem else 0)
        out.append((c0, w))
        c0 += w
    return out


def _build(C1, C2):
    """Per-core kernel: two experts (phase sizes C1 >= C2), 2-layer MLP."""
    nc = bacc.Bacc(None, target_bir_lowering=False)

    Cs = [C1, C2]
    ge0 = nc.dram_tensor("ge0", (P, 8, C1), BF16, kind="ExternalInput")
    ge1 = nc.dram_tensor("ge1", (P, 8, C2), BF16, kind="ExternalInput")
    # [ph, p(k_in), m, k, c]
    w1s = nc.dram_tensor("w1s", (NPH, P, 8, 8, P), BF16, kind="ExternalInput")
    # fp32 consts: cols 0:16 b1[m,ph], 16:32 w2[m,ph], 32:34 b2[ph]
    cst = nc.dram_tensor("cst", (P, 34), F32, kind="ExternalInput")
    w2s = nc.dram_tensor("w2s", (P, 8, NPH), BF16, kind="ExternalInput")
    out0 = nc.dram_tensor("out0", (1, C1), F32, kind="ExternalOutput")
    out1 = nc.dram_tensor("out1", (1, C2), F32, kind="ExternalOutput")

    ges = [ge0, ge1]
    outs = [out0, out1]
    chs = [_chunks(C1), _chunks(C2)]

    with tile.TileContext(nc) as tc:
        with (
            tc.tile_pool(name="const", bufs=1) as cpool,
            tc.tile_pool(name="ps1", bufs=2, space="PSUM") as ps1_pool,
            tc.tile_pool(name="ps2", bufs=2, space="PSUM") as ps2_pool,
            tc.tile_pool(name="psw", bufs=2, space="PSUM") as psw_pool,
        ):
            cst_sb = cpool.tile([P, 34], F32)
            w2_sb = cpool.tile([P, 8, NPH], BF16)
            ones = cpool.tile([P, 1], BF16)
            warm = cpool.tile([P, 512], BF16)
            emb_sb = [
                cpool.tile([P, 8, Cs[ph]], BF16, name=f"emb{ph}", tag=f"emb{ph}")
                for ph in range(NPH)
            ]
            w1_sb = cpool.tile([P, NPH, 8, 8, P], BF16)
            h_sb = [
                cpool.tile([P, 8, Cs[ph]], BF16, name=f"h{ph}", tag=f"h{ph}")
                for ph in range(NPH)
            ]
            acc_sb = [
                cpool.tile([P, Cs[ph]], BF16, name=f"acc{ph}", tag=f"acc{ph}")
                for ph in range(NPH)
            ]
            o_sb = [
                cpool.tile([1, Cs[ph]], F32, name=f"o{ph}", tag=f"o{ph}")
                for ph in range(NPH)
            ]

            nc.vector.memset(ones, 1.0)
            nc.vector.memset(warm, 0.0)

            # HAM warm-up: keep the PE busy while the first DMAs land so
            # the clock-gate is at 8/8 when real matmuls start.
            for i in range(6):
                pw = psw_pool.tile([P, 512], F32, name=f"pw{i}", tag="pw")
                nc.tensor.matmul(pw, warm[:, :128], warm, start=True, stop=True)

            # DMA triggers, ordered to feed the PE just-in-time.
            def dma_w1(ph, msl):
                nc.sync.dma_start(w1_sb[:, ph, msl], w1s[ph, :, msl])

            def dma_emb(ph, ci):
                c0, cw = chs[ph][ci]
                nc.sync.dma_start(
                    emb_sb[ph][:, :, c0:c0 + cw], ges[ph][:, :, c0:c0 + cw]
                )

            dma_w1(0, 0)
            nc.sync.dma_start(cst_sb, cst[:, :])
            nc.sync.dma_start(w2_sb, w2s[:, :, :])
            dma_emb(0, 0)
            dma_w1(0, 1)
            dma_w1(0, 2)
            for ci in range(1, len(chs[0])):
                dma_emb(0, ci)
            dma_w1(0, slice(3, 8))
            dma_w1(1, slice(0, 8))
            for ci in range(len(chs[1])):
                dma_emb(1, ci)

            pending = [None]

            def emit_p2(ph, ci, c0, cw):
                p2t = ps2_pool.tile(
                    [1, cw], F32, name=f"p2_{ph}_{ci}", tag="p2"
                )
                nc.tensor.matmul(
                    p2t, ones, acc_sb[ph][:, c0:c0 + cw], start=True, stop=False
                )
                nc.tensor.matmul(
                    p2t,
                    w2_sb[:, 7, ph:ph + 1],
                    h_sb[ph][:, 7, c0:c0 + cw],
                    start=False,
                    stop=True,
                )
                nc.scalar.activation(
                    o_sb[ph][:, c0:c0 + cw],
                    p2t,
                    mybir.ActivationFunctionType.Identity,
                    bias=cst_sb[0:1, 32 + ph:33 + ph],
                )
                if ci == len(chs[ph]) - 1:
                    nc.sync.dma_start(outs[ph][:, :], o_sb[ph])

            for ph in range(NPH):
                embT = emb_sb[ph]
                h = h_sb[ph]
                acc = acc_sb[ph]
                for ci, (c0, cw) in enumerate(chs[ph]):
                    for m in range(8):
                        p1 = ps1_pool.tile(
                            [P, cw], F32, name=f"p1_{ph}_{ci}_{m}", tag="p1"
                        )
                        for k in range(8):
                            nc.tensor.matmul(
                                p1,
                                w1_sb[:, ph, m, k],
                                embT[:, k, c0:c0 + cw],
                                start=(k == 0),
                                stop=(k == 7),
                            )
                        # previous chunk's L2 tail rides behind this
                        # chunk's first m-group so the PE never waits
                        # on the ScalarE/DVE chain.
                        if m == 0 and pending[0] is not None:
                            emit_p2(*pending[0])
                            pending[0] = None
                        bi = m * NPH + ph
                        nc.scalar.activation(
                            h[:, m, c0:c0 + cw],
                            p1,
                            mybir.ActivationFunctionType.Relu,
                            bias=cst_sb[:, bi:bi + 1],
                        )
                        if m == 0:
                            nc.vector.tensor_scalar_mul(
                                acc[:, c0:c0 + cw],
                                h[:, 0, c0:c0 + cw],
                                cst_sb[:, 16 + bi:17 + bi],
                            )
                        elif m < 7:
                            nc.vector.scalar_tensor_tensor(
                                acc[:, c0:c0 + cw],
                                h[:, m, c0:c0 + cw],
                                cst_sb[:, 16 + bi:17 + bi],
                                acc[:, c0:c0 + cw],
                                mybir.AluOpType.mult,
                                mybir.AluOpType.add,
                            )
                    pending[0] = (ph, ci, c0, cw)
            emit_p2(*pending[0])
    nc.finalize()
    return nc


def _route_host(emb, rw, rb):
    logits = emb.astype(np.float32) @ rw.astype(np.float32) + rb.astype(np.float32)
    i1 = np.argmax(logits, axis=1)
    l2m = logits.copy()
    l2m[np.arange(B), i1] = -np.inf
    i2 = np.argmax(l2m, axis=1)
    l1 = logits[np.arange(B), i1]
    l2 = l2m[np.arange(B), i2]
    d = np.exp(l2 - l1)
    wa = (1.0 / (1.0 + d)).astype(np.float32)
    wb = (1.0 - wa).astype(np.float32)
    comb = np.zeros((B, E), np.float32)
    comb[np.arange(B), i1] = wa
    comb[np.arange(B), i2] = wb
    return comb


def kernel(embeddings, router_w, router_b, w1, b1, w2, b2):
    emb = np.ascontiguousarray(np.asarray(embeddings, dtype=np.float32))
    rw = np.asarray(router_w, np.float32)
    rb = np.asarray(router_b, np.float32)
    w1 = np.asarray(w1, np.float32)
    b1 = np.asarray(b1, np.float32)
    w2 = np.asarray(w2, np.float32)
    b2 = np.asarray(b2, np.float32)

    comb = _route_host(emb, rw, rb)
    counts = (comb > 0).sum(axis=0)

    # count-aware schedule: 8 largest experts are phase 0 (one per core),
    # 8 smallest are phase 1; phase length = max count in the phase group.
    ranks = np.argsort(-counts, kind="stable")
    C1 = max(int(counts[ranks[0]]), 1)
    C2 = max(int(counts[ranks[8]]), 1)

    if (C1, C2) not in _CACHE:
        _CACHE[(C1, C2)] = _build(C1, C2)
    nc = _CACHE[(C1, C2)]

    embbf = emb.astype(BF_NP)

    in_maps = []
    toks = []  # per core, per phase: token ids
    for c in range(NCORES):
        es = [int(ranks[c]), int(ranks[8 + c])]
        ctoks = []
        ge_arrs = []
        for ph, e in enumerate(es):
            C = (C1, C2)[ph]
            ids = np.nonzero(comb[:, e] > 0)[0]
            ctoks.append(ids)
            g = np.zeros((C, H), BF_NP)
            g[: len(ids)] = embbf[ids]
            # [C, 8, 128] -> [128(p), 8(kb), C]
            ge_arrs.append(
                np.ascontiguousarray(g.reshape(C, 8, P).transpose(2, 1, 0))
            )
        toks.append(ctoks)
        # [ph, kb, p, mb, c] -> [ph, p, mb, kb, c]
        w1c = np.ascontiguousarray(
            w1[es].reshape(NPH, 8, P, 8, P).transpose(0, 2, 3, 1, 4).astype(BF_NP)
        )
        b1c = b1[es].reshape(NPH, 8, P).transpose(2, 1, 0)  # [P, m, ph]
        w2t = w2[es, :, 0].reshape(NPH, 8, P).transpose(2, 1, 0)  # [P, m, ph]
        cstc = np.empty((P, 34), np.float32)
        cstc[:, 0:16] = b1c.reshape(P, 16)
        cstc[:, 16:32] = w2t.reshape(P, 16)
        cstc[:, 32:34] = np.broadcast_to(b2[es, 0], (P, NPH))
        in_maps.append({
            "ge0": ge_arrs[0],
            "ge1": ge_arrs[1],
            "w1s": w1c,
            "cst": np.ascontiguousarray(cstc),
            "w2s": np.ascontiguousarray(w2t.astype(BF_NP)),
        })

    res = run_bass_kernel_spmd(nc, in_maps, core_ids=list(range(NCORES)))

    out = np.zeros((B,), np.float32)
    for c in range(NCORES):
        for ph, e in enumerate([int(ranks[c]), int(ranks[8 + c])]):
            ids = toks[c][ph]
            o = res.results[c][f"out{ph}"][0]
            out[ids] += comb[ids, e] * o[: len(ids)]
    return out.reshape(B, 1)
